# revision 4
# baseline (speedup 1.0000x reference)
"""Trainium2 Bass kernel for batched channel-attention (nn_Attention_28071906246667).

Reference computation (per batch element n, with xT = batch_flat[n] of shape [C, HW]):
    x   = xT.T                                  # [HW, C]
    Q   = x @ Wq.T + bq ; K, V likewise         # [HW, D]
    S   = Q.T @ K                               # [D, D]
    att = softmax(S, axis=-1)
    out = att @ V.T                             # [D, HW]

Key algebraic restructuring (halves FLOPs, avoids materializing Q/K/V):
    G = x.T x  (Gram over channels), m = column sums of x. Then
      S   = Wq G Wk.T + (Wq m) bk.T + bq (Wk m).T + HW bq bk.T
          = Wq_aug @ U,   U = [G m; m.T HW] @ WkT_aug
      out = att @ V.T = (att_unnorm @ Wv) @ xT + att_unnorm @ bv, normalized at the end.

Sharding: pure data parallel, batch N=16 -> 2 per core across 8 cores.
All matmuls run in float32r (fp32 with 11 explicit mantissa bits, full PE speed).
f32r ISA restrictions honored: moving operand & psum dst innermost counts even,
dst starts at partition 0.
"""

import numpy as np

N, C, HW, D = 16, 512, 3136, 512
NCORES = 8
NPC = N // NCORES          # batch elements per core
CT = C // 128              # 4 c partition tiles
DT = D // 128              # 4 d partition tiles
KT = 25                    # s k-tiles: 24 x 128 + 1 x 64
KT_ROWS = [128] * 24 + [64]
OC = 448                   # out-phase s chunk
NOC = HW // OC             # 7
IN_CH = 448                # input dma chunk along s
NIC = HW // IN_CH          # 7


def _f32r_round(a: np.ndarray) -> np.ndarray:
    """Round fp32 to float32r (11 explicit mantissa bits, round-to-nearest)."""
    bits = np.ascontiguousarray(a, dtype=np.float32).view(np.uint32)
    bits = (bits + np.uint32(0x800)) & np.uint32(0xFFFFF000)
    return bits.view(np.float32)


def _build_nc():
    import concourse.mybir as mybir
    from concourse import bacc
    from concourse.tile import TileContext

    f32 = mybir.dt.float32
    f32r = mybir.dt.float32r
    MUL = mybir.AluOpType.mult
    ADD = mybir.AluOpType.add

    nc = bacc.Bacc("TRN2", target_bir_lowering=False, debug=False,
                   num_devices=NCORES)

    x_ext = nc.declare_dram_parameter("x", [NPC, C, HW], f32r, isOutput=False)
    wqT_ext = nc.declare_dram_parameter("wqT", [C + 1, D], f32r, isOutput=False)
    wkT_ext = nc.declare_dram_parameter("wkT", [C + 1, D], f32r, isOutput=False)
    wv_ext = nc.declare_dram_parameter("wv", [D, C], f32r, isOutput=False)
    bv_ext = nc.declare_dram_parameter("bv2", [D, 2], f32r, isOutput=False)
    id_ext = nc.declare_dram_parameter("ident", [128, 128], f32r, isOutput=False)
    bkb_ext = nc.declare_dram_parameter("bkb", [128, D], f32r, isOutput=False)
    out_ext = nc.declare_dram_parameter("out", [NPC, D, HW], f32, isOutput=True)

    with TileContext(nc) as tc:
        with (
            tc.tile_pool(name="wpool", bufs=1) as wp,
            tc.tile_pool(name="xbig", bufs=2) as xb,
            tc.tile_pool(name="work", bufs=1) as wk,
            tc.tile_pool(name="small", bufs=2) as sm,
            tc.tile_pool(name="outsb", bufs=3) as osb,
            tc.tile_pool(name="acc4", bufs=4, space="PSUM") as acc4,
            tc.tile_pool(name="stage", bufs=3, space="PSUM") as stg,
            tc.tile_pool(name="extra", bufs=1, space="PSUM") as xtr,
        ):
            # ---- weights (loaded once) ----
            wq_t = wp.tile([128, CT, D], f32r, tag="wq")
            wq4 = wp.tile([1, D], f32r, tag="wq4")
            wk_t = wp.tile([128, CT, D], f32r, tag="wk")
            wk4 = wp.tile([1, D], f32r, tag="wk4")
            wv_t = wp.tile([128, DT, C], f32r, tag="wv")
            bv_t = wp.tile([128, DT, 2], f32r, tag="bv")
            ident = wp.tile([128, 128], f32r, tag="ident")
            bkb = wp.tile([128, D], f32r, tag="bkb")
            hw_t = wp.tile([1, 2], f32r, tag="hw")
            for k in range(CT):
                nc.sync.dma_start(out=wq_t[:, k, :], in_=wqT_ext[k * 128:(k + 1) * 128, :])
                nc.sync.dma_start(out=wk_t[:, k, :], in_=wkT_ext[k * 128:(k + 1) * 128, :])
                nc.sync.dma_start(out=wv_t[:, k, :], in_=wv_ext[k * 128:(k + 1) * 128, :])
                nc.sync.dma_start(out=bv_t[:, k, :], in_=bv_ext[k * 128:(k + 1) * 128, :])
            nc.sync.dma_start(out=wq4[:], in_=wqT_ext[C:C + 1, :])
            nc.sync.dma_start(out=wk4[:], in_=wkT_ext[C:C + 1, :])
            nc.sync.dma_start(out=ident[:], in_=id_ext[:])
            nc.sync.dma_start(out=bkb[:], in_=bkb_ext[:])
            nc.vector.memset(hw_t[:].bitcast(f32), float(HW))

            for n in range(NPC):
                # ---- load xT (c-major) ----
                xt = xb.tile([128, CT, HW], f32r, tag="xbig", name=f"xt{n}")
                for ci in range(CT):
                    for ch in range(NIC):
                        nc.sync.dma_start(
                            out=xt[:, ci, ch * IN_CH:(ch + 1) * IN_CH],
                            in_=x_ext[n, ci * 128:(ci + 1) * 128,
                                      ch * IN_CH:(ch + 1) * IN_CH])

                # ---- transpose to s-major + Gram accumulate ----
                xs = xb.tile([128, KT, C], f32r, tag="xbig", name=f"xs{n}")
                g_ps = [acc4.tile([128, 512], f32, tag="acc4", name=f"g_ps{n}_{j}")
                        for j in range(CT)]

                def emit_transp(kt):
                    rows = KT_ROWS[kt]
                    tp = stg.tile([128, 512], f32r, tag="stage", name=f"tp{n}_{kt}")
                    for cb in range(CT):
                        nc.tensor.transpose(
                            tp[:rows, cb * 128:(cb + 1) * 128],
                            xt[:, cb, kt * 128:kt * 128 + rows],
                            ident[:])
                    # evict to xs, alternating engines
                    if kt % 2 == 0:
                        nc.vector.tensor_copy(xs[:rows, kt, :], tp[:rows, :])
                    else:
                        nc.scalar.copy(xs[:rows, kt, :], tp[:rows, :])

                emit_transp(0)
                for kt in range(KT):
                    if kt + 1 < KT:
                        emit_transp(kt + 1)
                    rows = KT_ROWS[kt]
                    for j in range(CT):
                        nc.tensor.matmul(
                            g_ps[j][:],
                            xs[:rows, kt, j * 128:(j + 1) * 128],
                            xs[:rows, kt, :],
                            start=(kt == 0), stop=(kt == KT - 1))

                # ---- m = column sums of x (row sums of xT) ----
                m_f = sm.tile([128, CT], f32, tag="mf", name=f"mf{n}")
                m_r = sm.tile([128, CT], f32r, tag="mr", name=f"mr{n}")
                for ci in range(CT):
                    nc.vector.reduce_sum(m_f[:, ci:ci + 1], xt[:, ci, :],
                                         axis=mybir.AxisListType.X)
                nc.vector.tensor_copy(m_r[:], m_f[:])

                # ---- G to SBUF ----
                g = wk.tile([128, CT, 512], f32r, tag="g", name=f"g{n}")
                for j in range(CT):
                    nc.vector.tensor_copy(g[:, j, :], g_ps[j][:])

                # ---- U = G~ @ WkT_aug  [C+1, D] ----
                # rows 0..511: U[c,:] = sum_k G[k-tile, c] WkT[k] (+ m[c]*bk on evict)
                # row 512:     u4 = m.T @ WkT + HW*bk
                u = wk.tile([128, CT, D], f32r, tag="u", name=f"u{n}")
                u4 = wk.tile([1, D], f32r, tag="u4", name=f"u4{n}")
                u_ps = [acc4.tile([128, 512], f32, tag="acc4", name=f"u_ps{n}_{j}")
                        for j in range(CT)]
                u4_ps = xtr.tile([1, 512], f32, tag="extra", name=f"u4_ps{n}")
                for k in range(CT):
                    for j in range(CT):
                        nc.tensor.matmul(u_ps[j][:], g[:, k, j * 128:(j + 1) * 128],
                                         wk_t[:, k, :], start=(k == 0), stop=(k == CT - 1))
                    nc.tensor.matmul(u4_ps[:], m_r[:, k:k + 1], wk_t[:, k, :],
                                     start=(k == 0), stop=False)
                nc.tensor.matmul(u4_ps[:], hw_t[0:1, 0:1], wk4[:],
                                 start=False, stop=True)
                # evict with rank-1 bias update: u = u_ps + m[c] * bk[e]
                for j in range(CT):
                    nc.vector.scalar_tensor_tensor(
                        u[:, j, :], bkb[:], m_r[:, j:j + 1], u_ps[j][:],
                        op0=MUL, op1=ADD)
                nc.vector.tensor_copy(u4[:], u4_ps[:])

                # ---- S = Wq_aug @ U_aug ; softmax pieces ----
                s_ps = [acc4.tile([128, 512], f32, tag="acc4", name=f"s_ps{n}_{j}")
                        for j in range(DT)]
                for k in range(CT + 1):
                    lt = wq_t[:, k, :] if k < CT else wq4[:]
                    rhs = u[:, k, :] if k < CT else u4[:]
                    for jd in range(DT):
                        nc.tensor.matmul(s_ps[jd][:], lt[:, jd * 128:(jd + 1) * 128],
                                         rhs, start=(k == 0), stop=(k == CT))

                negmax = sm.tile([128, DT], f32, tag="negmax", name=f"negmax{n}")
                sumexp = sm.tile([128, DT], f32, tag="sumexp", name=f"sumexp{n}")
                recip = sm.tile([128, DT], f32, tag="recip", name=f"recip{n}")
                expS = wk.tile([128, DT, D], f32r, tag="expS", name=f"expS{n}")
                for jd in range(DT):
                    nc.vector.reduce_max(negmax[:, jd:jd + 1], s_ps[jd][:],
                                         axis=mybir.AxisListType.X, negate=True)
                    nc.scalar.activation(expS[:, jd, :], s_ps[jd][:],
                                         mybir.ActivationFunctionType.Exp,
                                         bias=negmax[:, jd:jd + 1], scale=1.0,
                                         accum_out=sumexp[:, jd:jd + 1])
                nc.vector.reciprocal(recip[:], sumexp[:])

                # ---- attT via transposes ----
                attT = wk.tile([128, DT, D], f32r, tag="attT", name=f"attT{n}")
                for je in range(DT):
                    at = stg.tile([128, 512], f32r, tag="stage", name=f"at{n}_{je}")
                    for jd in range(DT):
                        nc.tensor.transpose(at[:, jd * 128:(jd + 1) * 128],
                                            expS[:, jd, je * 128:(je + 1) * 128],
                                            ident[:])
                    nc.vector.tensor_copy(attT[:, je, :], at[:])

                # ---- B^T = Wv^T @ attT ; bias_d = expS @ bv (N=2 padded) ----
                bT = wk.tile([128, CT, D], f32r, tag="bT", name=f"bT{n}")
                b_ps = [acc4.tile([128, 512], f32, tag="acc4", name=f"b_ps{n}_{j}")
                        for j in range(CT)]
                bias_ps = xtr.tile([128, 8], f32, tag="extra", name=f"bias_ps{n}")
                for je in range(DT):
                    for jc in range(CT):
                        nc.tensor.matmul(b_ps[jc][:], wv_t[:, je, jc * 128:(jc + 1) * 128],
                                         attT[:, je, :], start=(je == 0), stop=(je == DT - 1))
                    for jd in range(DT):
                        nc.tensor.matmul(bias_ps[:, 2 * jd:2 * jd + 2],
                                         attT[:, je, jd * 128:(jd + 1) * 128],
                                         bv_t[:, je, :],
                                         start=(je == 0), stop=(je == DT - 1))
                for jc in range(CT):
                    nc.vector.tensor_copy(bT[:, jc, :], b_ps[jc][:])
                bias_eff = sm.tile([128, DT], f32, tag="bias_eff", name=f"bias_eff{n}")
                for jd in range(DT):
                    nc.vector.tensor_mul(bias_eff[:, jd:jd + 1],
                                         bias_ps[:, 2 * jd:2 * jd + 1],
                                         recip[:, jd:jd + 1])

                # ---- out = B^T.T @ xT, scaled+biased on eviction ----
                for jd in range(DT):
                    for ch in range(NOC):
                        o_ps = stg.tile([128, OC], f32, tag="stage",
                                        name=f"o_ps{n}_{jd}_{ch}")
                        for k in range(CT):
                            nc.tensor.matmul(o_ps[:], bT[:, k, jd * 128:(jd + 1) * 128],
                                             xt[:, k, ch * OC:(ch + 1) * OC],
                                             start=(k == 0), stop=(k == CT - 1))
                        o_sb = osb.tile([128, OC], f32, tag="osb",
                                        name=f"o_sb{n}_{jd}_{ch}")
                        nc.scalar.activation(o_sb[:], o_ps[:],
                                             mybir.ActivationFunctionType.Identity,
                                             bias=bias_eff[:, jd:jd + 1],
                                             scale=recip[:, jd:jd + 1])
                        nc.sync.dma_start(
                            out=out_ext[n, jd * 128:(jd + 1) * 128,
                                        ch * OC:(ch + 1) * OC],
                            in_=o_sb[:])

    nc.compile()
    return nc


_NC_CACHE = None


def kernel(**inputs: np.ndarray) -> np.ndarray:
    global _NC_CACHE
    from concourse.bass_utils import run_bass_kernel_spmd

    batch = np.asarray(inputs["batch_flat"], dtype=np.float32)
    Wq = np.asarray(inputs["Wq"], dtype=np.float32)
    bq = np.asarray(inputs["bq"], dtype=np.float32)
    Wk = np.asarray(inputs["Wk"], dtype=np.float32)
    bk = np.asarray(inputs["bk"], dtype=np.float32)
    Wv = np.asarray(inputs["Wv"], dtype=np.float32)
    bv = np.asarray(inputs["bv"], dtype=np.float32)

    if _NC_CACHE is None:
        _NC_CACHE = _build_nc()
    nc = _NC_CACHE

    x_r = _f32r_round(batch)
    wqT = _f32r_round(np.concatenate([Wq.T, bq[None, :]], axis=0))
    wkT = _f32r_round(np.concatenate([Wk.T, bk[None, :]], axis=0))
    wv = _f32r_round(Wv)
    bv2 = np.zeros((D, 2), dtype=np.float32)
    bv2[:, 0] = _f32r_round(bv)
    ident = np.eye(128, dtype=np.float32)

    in_maps = []
    for c in range(NCORES):
        in_maps.append({
            "x": np.ascontiguousarray(x_r[c * NPC:(c + 1) * NPC]),
            "wqT": wqT, "wkT": wkT, "wv": wv, "bv2": bv2, "ident": ident,
            "bkb": np.ascontiguousarray(np.tile(wkT[C:C + 1, :], (128, 1))),
        })
    r = run_bass_kernel_spmd(nc, in_maps, core_ids=list(range(NCORES)))
    out = np.concatenate([r.results[c]["out"] for c in range(NCORES)], axis=0)
    return out.astype(np.float32)


# revision 5
# speedup vs baseline: 1.0220x; 1.0220x over previous
"""Trainium2 Bass kernel for batched channel-attention (nn_Attention_28071906246667).

Reference computation (per batch element n, with xT = batch_flat[n] of shape [C, HW]):
    x   = xT.T                                  # [HW, C]
    Q   = x @ Wq.T + bq ; K, V likewise         # [HW, D]
    S   = Q.T @ K                               # [D, D]
    att = softmax(S, axis=-1)
    out = att @ V.T                             # [D, HW]

Key algebraic restructuring (halves FLOPs, avoids materializing Q/K/V):
    G = x.T x  (Gram over channels), m = column sums of x. Then
      S   = Wq G Wk.T + (Wq m) bk.T + bq (Wk m).T + HW bq bk.T
          = Wq_aug @ U,   U = [G m; m.T HW] @ WkT_aug
      out = att @ V.T = (att_unnorm @ Wv) @ xT + att_unnorm @ bv, normalized at the end.

Sharding: pure data parallel, batch N=16 -> 2 per core across 8 cores.
All matmuls run in float32r (fp32 with 11 explicit mantissa bits, full PE speed).
f32r ISA restrictions honored: moving operand & psum dst innermost counts even,
dst starts at partition 0.
"""

import numpy as np

N, C, HW, D = 16, 512, 3136, 512
NCORES = 8
NPC = N // NCORES          # batch elements per core
CT = C // 128              # 4 c partition tiles
DT = D // 128              # 4 d partition tiles
KT = 25                    # s k-tiles: 24 x 128 + 1 x 64
KT_ROWS = [128] * 24 + [64]
OC = 448                   # out-phase s chunk
NOC = HW // OC             # 7
IN_CH = 448                # input dma chunk along s
NIC = HW // IN_CH          # 7


def _f32r_round(a: np.ndarray) -> np.ndarray:
    """Round fp32 to float32r (11 explicit mantissa bits, round-to-nearest)."""
    bits = np.ascontiguousarray(a, dtype=np.float32).view(np.uint32)
    bits = (bits + np.uint32(0x800)) & np.uint32(0xFFFFF000)
    return bits.view(np.float32)


def _build_nc():
    import concourse.mybir as mybir
    from concourse import bacc
    from concourse.tile import TileContext

    f32 = mybir.dt.float32
    f32r = mybir.dt.float32r
    MUL = mybir.AluOpType.mult
    ADD = mybir.AluOpType.add

    nc = bacc.Bacc("TRN2", target_bir_lowering=False, debug=False,
                   num_devices=NCORES)

    x_ext = nc.declare_dram_parameter("x", [NPC, C, HW], f32r, isOutput=False)
    wqT_ext = nc.declare_dram_parameter("wqT", [C + 1, D], f32r, isOutput=False)
    wkT_ext = nc.declare_dram_parameter("wkT", [C + 1, D], f32r, isOutput=False)
    wv_ext = nc.declare_dram_parameter("wv", [D, C], f32r, isOutput=False)
    id_ext = nc.declare_dram_parameter("ident", [128, 128], f32r, isOutput=False)
    bkb_ext = nc.declare_dram_parameter("bkb", [128, D], f32r, isOutput=False)
    bvb_ext = nc.declare_dram_parameter("bvb", [128, D], f32r, isOutput=False)
    out_ext = nc.declare_dram_parameter("out", [NPC, D, HW], f32, isOutput=True)

    with TileContext(nc) as tc:
        with (
            tc.tile_pool(name="wpool", bufs=1) as wp,
            tc.tile_pool(name="xbig", bufs=2) as xb,
            tc.tile_pool(name="work", bufs=1) as wk,
            tc.tile_pool(name="small", bufs=2) as sm,
            tc.tile_pool(name="outsb", bufs=3) as osb,
            tc.tile_pool(name="acc4", bufs=4, space="PSUM") as acc4,
            tc.tile_pool(name="stage", bufs=3, space="PSUM") as stg,
            tc.tile_pool(name="extra", bufs=1, space="PSUM") as xtr,
        ):
            # ---- weights (loaded once) ----
            wq_t = wp.tile([128, CT, D], f32r, tag="wq")
            wq4 = wp.tile([1, D], f32r, tag="wq4")
            wk_t = wp.tile([128, CT, D], f32r, tag="wk")
            wk4 = wp.tile([1, D], f32r, tag="wk4")
            wv_t = wp.tile([128, DT, C], f32r, tag="wv")
            ident = wp.tile([128, 128], f32r, tag="ident")
            bkb = wp.tile([128, D], f32r, tag="bkb")
            bvb = wp.tile([128, D], f32r, tag="bvb")
            hw_t = wp.tile([1, 2], f32r, tag="hw")
            for k in range(CT):
                nc.sync.dma_start(out=wq_t[:, k, :], in_=wqT_ext[k * 128:(k + 1) * 128, :])
                nc.sync.dma_start(out=wk_t[:, k, :], in_=wkT_ext[k * 128:(k + 1) * 128, :])
                nc.sync.dma_start(out=wv_t[:, k, :], in_=wv_ext[k * 128:(k + 1) * 128, :])
            nc.sync.dma_start(out=wq4[:], in_=wqT_ext[C:C + 1, :])
            nc.sync.dma_start(out=wk4[:], in_=wkT_ext[C:C + 1, :])
            nc.sync.dma_start(out=ident[:], in_=id_ext[:])
            nc.sync.dma_start(out=bkb[:], in_=bkb_ext[:])
            nc.sync.dma_start(out=bvb[:], in_=bvb_ext[:])
            nc.vector.memset(hw_t[:].bitcast(f32), float(HW))

            for n in range(NPC):
                # ---- load xT (c-major) ----
                xt = xb.tile([128, CT, HW], f32r, tag="xbig", name=f"xt{n}")
                for ci in range(CT):
                    for ch in range(NIC):
                        nc.sync.dma_start(
                            out=xt[:, ci, ch * IN_CH:(ch + 1) * IN_CH],
                            in_=x_ext[n, ci * 128:(ci + 1) * 128,
                                      ch * IN_CH:(ch + 1) * IN_CH])

                # ---- transpose to s-major + Gram accumulate ----
                xs = xb.tile([128, KT, C], f32r, tag="xbig", name=f"xs{n}")
                g_ps = [acc4.tile([128, 512], f32, tag="acc4", name=f"g_ps{n}_{j}")
                        for j in range(CT)]

                def emit_transp(kt):
                    rows = KT_ROWS[kt]
                    tp = stg.tile([128, 512], f32r, tag="stage", name=f"tp{n}_{kt}")
                    for cb in range(CT):
                        nc.tensor.transpose(
                            tp[:rows, cb * 128:(cb + 1) * 128],
                            xt[:, cb, kt * 128:kt * 128 + rows],
                            ident[:])
                    # evict to xs, alternating engines
                    if kt % 2 == 0:
                        nc.vector.tensor_copy(xs[:rows, kt, :], tp[:rows, :])
                    else:
                        nc.scalar.copy(xs[:rows, kt, :], tp[:rows, :])

                emit_transp(0)
                for kt in range(KT):
                    if kt + 1 < KT:
                        emit_transp(kt + 1)
                    rows = KT_ROWS[kt]
                    for j in range(CT):
                        nc.tensor.matmul(
                            g_ps[j][:],
                            xs[:rows, kt, j * 128:(j + 1) * 128],
                            xs[:rows, kt, :],
                            start=(kt == 0), stop=(kt == KT - 1))

                # ---- m = column sums of x (row sums of xT) ----
                m_f = sm.tile([128, CT], f32, tag="mf", name=f"mf{n}")
                m_r = sm.tile([128, CT], f32r, tag="mr", name=f"mr{n}")
                for ci in range(CT):
                    nc.vector.reduce_sum(m_f[:, ci:ci + 1], xt[:, ci, :],
                                         axis=mybir.AxisListType.X)
                nc.vector.tensor_copy(m_r[:], m_f[:])

                # ---- G to SBUF ----
                g = wk.tile([128, CT, 512], f32r, tag="g", name=f"g{n}")
                for j in range(CT):
                    nc.vector.tensor_copy(g[:, j, :], g_ps[j][:])

                # ---- U = G~ @ WkT_aug  [C+1, D] ----
                # rows 0..511: U[c,:] = sum_k G[k-tile, c] WkT[k] (+ m[c]*bk on evict)
                # row 512:     u4 = m.T @ WkT + HW*bk
                u = wk.tile([128, CT, D], f32r, tag="u", name=f"u{n}")
                u4 = wk.tile([1, D], f32r, tag="u4", name=f"u4{n}")
                u_ps = [acc4.tile([128, 512], f32, tag="acc4", name=f"u_ps{n}_{j}")
                        for j in range(CT)]
                u4_ps = xtr.tile([1, 512], f32, tag="extra", name=f"u4_ps{n}")
                for k in range(CT):
                    for j in range(CT):
                        nc.tensor.matmul(u_ps[j][:], g[:, k, j * 128:(j + 1) * 128],
                                         wk_t[:, k, :], start=(k == 0), stop=(k == CT - 1))
                    nc.tensor.matmul(u4_ps[:], m_r[:, k:k + 1], wk_t[:, k, :],
                                     start=(k == 0), stop=False)
                nc.tensor.matmul(u4_ps[:], hw_t[0:1, 0:1], wk4[:],
                                 start=False, stop=True)
                # evict with rank-1 bias update: u = u_ps + m[c] * bk[e]
                for j in range(CT):
                    nc.vector.scalar_tensor_tensor(
                        u[:, j, :], bkb[:], m_r[:, j:j + 1], u_ps[j][:],
                        op0=MUL, op1=ADD)
                nc.vector.tensor_copy(u4[:], u4_ps[:])

                # ---- S = Wq_aug @ U_aug ; softmax pieces ----
                s_ps = [acc4.tile([128, 512], f32, tag="acc4", name=f"s_ps{n}_{j}")
                        for j in range(DT)]
                for k in range(CT + 1):
                    lt = wq_t[:, k, :] if k < CT else wq4[:]
                    rhs = u[:, k, :] if k < CT else u4[:]
                    for jd in range(DT):
                        nc.tensor.matmul(s_ps[jd][:], lt[:, jd * 128:(jd + 1) * 128],
                                         rhs, start=(k == 0), stop=(k == CT))

                negmax = sm.tile([128, DT], f32, tag="negmax", name=f"negmax{n}")
                sumexp = sm.tile([128, DT], f32, tag="sumexp", name=f"sumexp{n}")
                recip = sm.tile([128, DT], f32, tag="recip", name=f"recip{n}")
                expS = wk.tile([128, DT, D], f32r, tag="expS", name=f"expS{n}")
                for jd in range(DT):
                    nc.vector.reduce_max(negmax[:, jd:jd + 1], s_ps[jd][:],
                                         axis=mybir.AxisListType.X, negate=True)
                    nc.scalar.activation(expS[:, jd, :], s_ps[jd][:],
                                         mybir.ActivationFunctionType.Exp,
                                         bias=negmax[:, jd:jd + 1], scale=1.0,
                                         accum_out=sumexp[:, jd:jd + 1])
                nc.vector.reciprocal(recip[:], sumexp[:])
                bias_d = sm.tile([128, DT], f32, tag="bias_d", name=f"bias_d{n}")
                bias_dummy = sm.tile([128, 512], f32, tag="bias_dummy",
                                     name=f"bias_dummy{n}")
                for jd in range(DT):
                    nc.vector.scalar_tensor_tensor(
                        bias_dummy[:], expS[:, jd, :], 1.0, bvb[:],
                        op0=MUL, op1=MUL, accum_out=bias_d[:, jd:jd + 1])

                # ---- attT via transposes ----
                attT = wk.tile([128, DT, D], f32r, tag="attT", name=f"attT{n}")
                for je in range(DT):
                    at = stg.tile([128, 512], f32r, tag="stage", name=f"at{n}_{je}")
                    for jd in range(DT):
                        nc.tensor.transpose(at[:, jd * 128:(jd + 1) * 128],
                                            expS[:, jd, je * 128:(je + 1) * 128],
                                            ident[:])
                    nc.vector.tensor_copy(attT[:, je, :], at[:])

                # ---- B^T = Wv^T @ attT ; bias_d = expS @ bv (N=2 padded) ----
                bT = wk.tile([128, CT, D], f32r, tag="bT", name=f"bT{n}")
                b_ps = [acc4.tile([128, 512], f32, tag="acc4", name=f"b_ps{n}_{j}")
                        for j in range(CT)]
                for je in range(DT):
                    for jc in range(CT):
                        nc.tensor.matmul(b_ps[jc][:], wv_t[:, je, jc * 128:(jc + 1) * 128],
                                         attT[:, je, :], start=(je == 0), stop=(je == DT - 1))
                for jc in range(CT):
                    nc.vector.tensor_copy(bT[:, jc, :], b_ps[jc][:])
                bias_eff = sm.tile([128, DT], f32, tag="bias_eff", name=f"bias_eff{n}")
                nc.vector.tensor_mul(bias_eff[:], bias_d[:], recip[:])

                # ---- out = B^T.T @ xT, scaled+biased on eviction ----
                for jd in range(DT):
                    for ch in range(NOC):
                        o_ps = stg.tile([128, OC], f32, tag="stage",
                                        name=f"o_ps{n}_{jd}_{ch}")
                        for k in range(CT):
                            nc.tensor.matmul(o_ps[:], bT[:, k, jd * 128:(jd + 1) * 128],
                                             xt[:, k, ch * OC:(ch + 1) * OC],
                                             start=(k == 0), stop=(k == CT - 1))
                        o_sb = osb.tile([128, OC], f32, tag="osb",
                                        name=f"o_sb{n}_{jd}_{ch}")
                        nc.scalar.activation(o_sb[:], o_ps[:],
                                             mybir.ActivationFunctionType.Identity,
                                             bias=bias_eff[:, jd:jd + 1],
                                             scale=recip[:, jd:jd + 1])
                        nc.sync.dma_start(
                            out=out_ext[n, jd * 128:(jd + 1) * 128,
                                        ch * OC:(ch + 1) * OC],
                            in_=o_sb[:])

    nc.compile()
    return nc


_NC_CACHE = None


def kernel(**inputs: np.ndarray) -> np.ndarray:
    global _NC_CACHE
    from concourse.bass_utils import run_bass_kernel_spmd

    batch = np.asarray(inputs["batch_flat"], dtype=np.float32)
    Wq = np.asarray(inputs["Wq"], dtype=np.float32)
    bq = np.asarray(inputs["bq"], dtype=np.float32)
    Wk = np.asarray(inputs["Wk"], dtype=np.float32)
    bk = np.asarray(inputs["bk"], dtype=np.float32)
    Wv = np.asarray(inputs["Wv"], dtype=np.float32)
    bv = np.asarray(inputs["bv"], dtype=np.float32)

    if _NC_CACHE is None:
        _NC_CACHE = _build_nc()
    nc = _NC_CACHE

    x_r = _f32r_round(batch)
    wqT = _f32r_round(np.concatenate([Wq.T, bq[None, :]], axis=0))
    wkT = _f32r_round(np.concatenate([Wk.T, bk[None, :]], axis=0))
    wv = _f32r_round(Wv)
    bvb = np.ascontiguousarray(np.tile(_f32r_round(bv)[None, :], (128, 1)))
    ident = np.eye(128, dtype=np.float32)

    in_maps = []
    for c in range(NCORES):
        in_maps.append({
            "x": np.ascontiguousarray(x_r[c * NPC:(c + 1) * NPC]),
            "wqT": wqT, "wkT": wkT, "wv": wv, "bvb": bvb, "ident": ident,
            "bkb": np.ascontiguousarray(np.tile(wkT[C:C + 1, :], (128, 1))),
        })
    r = run_bass_kernel_spmd(nc, in_maps, core_ids=list(range(NCORES)))
    out = np.concatenate([r.results[c]["out"] for c in range(NCORES)], axis=0)
    return out.astype(np.float32)


# revision 6
# speedup vs baseline: 1.1294x; 1.1051x over previous
"""Trainium2 Bass kernel for batched channel-attention (nn_Attention_28071906246667).

Reference computation (per batch element n, with xT = batch_flat[n] of shape [C, HW]):
    x   = xT.T                                  # [HW, C]
    Q   = x @ Wq.T + bq ; K, V likewise         # [HW, D]
    S   = Q.T @ K                               # [D, D]
    att = softmax(S, axis=-1)
    out = att @ V.T                             # [D, HW]

Key algebraic restructuring (halves FLOPs, avoids materializing Q/K/V):
    G = x.T x  (Gram over channels), m = column sums of x. Then
      S   = Wq G Wk.T + (Wq m) bk.T + bq (Wk m).T + HW bq bk.T
          = Wq_aug @ U,   U = [G m; m.T HW] @ WkT_aug
      out = att @ V.T = (att_unnorm @ Wv) @ xT + att_unnorm @ bv, normalized at the end.

Sharding: pure data parallel, batch N=16 -> 2 per core across 8 cores.
All matmuls run in float32r (fp32 with 11 explicit mantissa bits, full PE speed).
f32r ISA restrictions honored: moving operand & psum dst innermost counts even,
dst starts at partition 0.
"""

import numpy as np

N, C, HW, D = 16, 512, 3136, 512
NCORES = 8
NPC = N // NCORES          # batch elements per core
CT = C // 128              # 4 c partition tiles
DT = D // 128              # 4 d partition tiles
KT = 25                    # s k-tiles: 24 x 128 + 1 x 64
KT_ROWS = [128] * 24 + [64]
OC = 448                   # out-phase s chunk
NOC = HW // OC             # 7
IN_CH = 448                # input dma chunk along s
NIC = HW // IN_CH          # 7


def _f32r_round(a: np.ndarray) -> np.ndarray:
    """Round fp32 to float32r (11 explicit mantissa bits, round-to-nearest)."""
    bits = np.ascontiguousarray(a, dtype=np.float32).view(np.uint32)
    bits = (bits + np.uint32(0x800)) & np.uint32(0xFFFFF000)
    return bits.view(np.float32)


def _build_nc():
    import concourse.mybir as mybir
    from concourse import bacc
    from concourse.tile import TileContext

    f32 = mybir.dt.float32
    f32r = mybir.dt.float32r
    MUL = mybir.AluOpType.mult
    ADD = mybir.AluOpType.add

    nc = bacc.Bacc("TRN2", target_bir_lowering=False, debug=False,
                   num_devices=NCORES)

    x_ext = nc.declare_dram_parameter("x", [NPC, C, HW], f32r, isOutput=False)
    wqT_ext = nc.declare_dram_parameter("wqT", [C + 1, D], f32r, isOutput=False)
    wkT_ext = nc.declare_dram_parameter("wkT", [C + 1, D], f32r, isOutput=False)
    wv_ext = nc.declare_dram_parameter("wv", [D, C], f32r, isOutput=False)
    id_ext = nc.declare_dram_parameter("ident", [128, 128], f32r, isOutput=False)
    bkb_ext = nc.declare_dram_parameter("bkb", [128, D], f32r, isOutput=False)
    bvb_ext = nc.declare_dram_parameter("bvb", [128, D], f32r, isOutput=False)
    out_ext = nc.declare_dram_parameter("out", [NPC, D, HW], f32, isOutput=True)

    with TileContext(nc) as tc:
        with (
            tc.tile_pool(name="wpool", bufs=1) as wp,
            tc.tile_pool(name="xbig", bufs=2) as xb,
            tc.tile_pool(name="work", bufs=1) as wk,
            tc.tile_pool(name="small", bufs=2) as sm,
            tc.tile_pool(name="outsb", bufs=3) as osb,
            tc.tile_pool(name="acc4", bufs=4, space="PSUM") as acc4,
            tc.tile_pool(name="stage", bufs=3, space="PSUM") as stg,
            tc.tile_pool(name="extra", bufs=1, space="PSUM") as xtr,
        ):
            # ---- weights (loaded once) ----
            wq_t = wp.tile([128, CT, D], f32r, tag="wq")
            wq4 = wp.tile([1, D], f32r, tag="wq4")
            wk_t = wp.tile([128, CT, D], f32r, tag="wk")
            wk4 = wp.tile([1, D], f32r, tag="wk4")
            wv_t = wp.tile([128, DT, C], f32r, tag="wv")
            ident = wp.tile([128, 128], f32r, tag="ident")
            bkb = wp.tile([128, D], f32r, tag="bkb")
            bvb = wp.tile([128, D], f32r, tag="bvb")
            hw_t = wp.tile([1, 2], f32r, tag="hw")
            for k in range(CT):
                nc.sync.dma_start(out=wq_t[:, k, :], in_=wqT_ext[k * 128:(k + 1) * 128, :])
                nc.sync.dma_start(out=wk_t[:, k, :], in_=wkT_ext[k * 128:(k + 1) * 128, :])
                nc.sync.dma_start(out=wv_t[:, k, :], in_=wv_ext[k * 128:(k + 1) * 128, :])
            nc.sync.dma_start(out=wq4[:], in_=wqT_ext[C:C + 1, :])
            nc.sync.dma_start(out=wk4[:], in_=wkT_ext[C:C + 1, :])
            nc.sync.dma_start(out=ident[:], in_=id_ext[:])
            nc.sync.dma_start(out=bkb[:], in_=bkb_ext[:])
            nc.sync.dma_start(out=bvb[:], in_=bvb_ext[:])
            nc.vector.memset(hw_t[:].bitcast(f32), float(HW))

            for n in range(NPC):
                # ---- load xT (c-major) ----
                xt = xb.tile([128, CT, HW], f32r, tag="xbig", name=f"xt{n}")
                for ch in range(NIC):
                    for ci in range(CT):
                        nc.sync.dma_start(
                            out=xt[:, ci, ch * IN_CH:(ch + 1) * IN_CH],
                            in_=x_ext[n, ci * 128:(ci + 1) * 128,
                                      ch * IN_CH:(ch + 1) * IN_CH])

                # ---- transpose to s-major + Gram accumulate ----
                xs = xb.tile([128, KT, C], f32r, tag="xbig", name=f"xs{n}")
                g_ps = [acc4.tile([128, 512], f32, tag="acc4", name=f"g_ps{n}_{j}")
                        for j in range(CT)]

                def emit_transp(kt):
                    rows = KT_ROWS[kt]
                    tp = stg.tile([128, 512], f32r, tag="stage", name=f"tp{n}_{kt}")
                    for cb in range(CT):
                        nc.tensor.transpose(
                            tp[:rows, cb * 128:(cb + 1) * 128],
                            xt[:, cb, kt * 128:kt * 128 + rows],
                            ident[:])
                    nc.scalar.copy(xs[:rows, kt, :], tp[:rows, :])

                emit_transp(0)
                for kt in range(KT):
                    if kt + 1 < KT:
                        emit_transp(kt + 1)
                    rows = KT_ROWS[kt]
                    for j in range(CT):
                        nc.tensor.matmul(
                            g_ps[j][:],
                            xs[:rows, kt, j * 128:(j + 1) * 128],
                            xs[:rows, kt, :],
                            start=(kt == 0), stop=(kt == KT - 1))

                # ---- m = column sums of x (row sums of xT) ----
                m_f = sm.tile([128, CT], f32, tag="mf", name=f"mf{n}")
                m_r = sm.tile([128, CT], f32r, tag="mr", name=f"mr{n}")
                for ci in range(CT):
                    nc.vector.reduce_sum(m_f[:, ci:ci + 1], xt[:, ci, :],
                                         axis=mybir.AxisListType.X)
                nc.vector.tensor_copy(m_r[:], m_f[:])

                # ---- G to SBUF ----
                g = wk.tile([128, CT, 512], f32r, tag="g", name=f"g{n}")
                for j in range(CT):
                    nc.vector.tensor_copy(g[:, j, :], g_ps[j][:])

                # ---- U = G~ @ WkT_aug  [C+1, D] ----
                # rows 0..511: U[c,:] = sum_k G[k-tile, c] WkT[k] (+ m[c]*bk on evict)
                # row 512:     u4 = m.T @ WkT + HW*bk
                u = wk.tile([128, CT, D], f32r, tag="u", name=f"u{n}")
                u4 = wk.tile([1, D], f32r, tag="u4", name=f"u4{n}")
                u_ps = [acc4.tile([128, 512], f32, tag="acc4", name=f"u_ps{n}_{j}")
                        for j in range(CT)]
                u4_ps = xtr.tile([1, 512], f32, tag="extra", name=f"u4_ps{n}")
                for k in range(CT):
                    for j in range(CT):
                        nc.tensor.matmul(u_ps[j][:], g[:, k, j * 128:(j + 1) * 128],
                                         wk_t[:, k, :], start=(k == 0), stop=(k == CT - 1))
                for k in range(CT):
                    nc.tensor.matmul(u4_ps[:], m_r[:, k:k + 1], wk_t[:, k, :],
                                     start=(k == 0), stop=False)
                nc.tensor.matmul(u4_ps[:], hw_t[0:1, 0:1], wk4[:],
                                 start=False, stop=True)
                # evict with rank-1 bias update: u = u_ps + m[c] * bk[e]
                for j in range(CT):
                    nc.vector.scalar_tensor_tensor(
                        u[:, j, :], bkb[:], m_r[:, j:j + 1], u_ps[j][:],
                        op0=MUL, op1=ADD)
                nc.vector.tensor_copy(u4[:], u4_ps[:])

                # ---- S = Wq_aug @ U_aug ; softmax pieces ----
                s_ps = [acc4.tile([128, 512], f32, tag="acc4", name=f"s_ps{n}_{j}")
                        for j in range(DT)]
                for k in range(CT + 1):
                    lt = wq_t[:, k, :] if k < CT else wq4[:]
                    rhs = u[:, k, :] if k < CT else u4[:]
                    for jd in range(DT):
                        nc.tensor.matmul(s_ps[jd][:], lt[:, jd * 128:(jd + 1) * 128],
                                         rhs, start=(k == 0), stop=(k == CT))

                negmax = sm.tile([128, DT], f32, tag="negmax", name=f"negmax{n}")
                sumexp = sm.tile([128, DT], f32, tag="sumexp", name=f"sumexp{n}")
                recip = sm.tile([128, DT], f32, tag="recip", name=f"recip{n}")
                expS = wk.tile([128, DT, D], f32r, tag="expS", name=f"expS{n}")
                for jd in range(DT):
                    nc.vector.reduce_max(negmax[:, jd:jd + 1], s_ps[jd][:],
                                         axis=mybir.AxisListType.X, negate=True)
                    nc.scalar.activation(expS[:, jd, :], s_ps[jd][:],
                                         mybir.ActivationFunctionType.Exp,
                                         bias=negmax[:, jd:jd + 1], scale=1.0,
                                         accum_out=sumexp[:, jd:jd + 1])
                nc.vector.reciprocal(recip[:], sumexp[:])
                bias_d = sm.tile([128, DT], f32, tag="bias_d", name=f"bias_d{n}")
                bias_dummy = sm.tile([128, 512], f32, tag="bias_dummy",
                                     name=f"bias_dummy{n}")
                for jd in range(DT):
                    nc.vector.scalar_tensor_tensor(
                        bias_dummy[:], expS[:, jd, :], 1.0, bvb[:],
                        op0=MUL, op1=MUL, accum_out=bias_d[:, jd:jd + 1])

                # ---- attT via transposes ----
                attT = wk.tile([128, DT, D], f32r, tag="attT", name=f"attT{n}")
                for je in range(DT):
                    at = stg.tile([128, 512], f32r, tag="stage", name=f"at{n}_{je}")
                    for jd in range(DT):
                        nc.tensor.transpose(at[:, jd * 128:(jd + 1) * 128],
                                            expS[:, jd, je * 128:(je + 1) * 128],
                                            ident[:])
                    nc.vector.tensor_copy(attT[:, je, :], at[:])

                # ---- B^T = Wv^T @ attT ; bias_d = expS @ bv (N=2 padded) ----
                bT = wk.tile([128, CT, D], f32r, tag="bT", name=f"bT{n}")
                b_ps = [acc4.tile([128, 512], f32, tag="acc4", name=f"b_ps{n}_{j}")
                        for j in range(CT)]
                for je in range(DT):
                    for jc in range(CT):
                        nc.tensor.matmul(b_ps[jc][:], wv_t[:, je, jc * 128:(jc + 1) * 128],
                                         attT[:, je, :], start=(je == 0), stop=(je == DT - 1))
                for jc in range(CT):
                    nc.vector.tensor_copy(bT[:, jc, :], b_ps[jc][:])
                bias_eff = sm.tile([128, DT], f32, tag="bias_eff", name=f"bias_eff{n}")
                nc.vector.tensor_mul(bias_eff[:], bias_d[:], recip[:])

                # ---- out = B^T.T @ xT, scaled+biased on eviction ----
                for jd in range(DT):
                    for ch in range(NOC):
                        o_ps = stg.tile([128, OC], f32, tag="stage",
                                        name=f"o_ps{n}_{jd}_{ch}")
                        for k in range(CT):
                            nc.tensor.matmul(o_ps[:], bT[:, k, jd * 128:(jd + 1) * 128],
                                             xt[:, k, ch * OC:(ch + 1) * OC],
                                             start=(k == 0), stop=(k == CT - 1))
                        o_sb = osb.tile([128, OC], f32, tag="osb",
                                        name=f"o_sb{n}_{jd}_{ch}")
                        nc.scalar.activation(o_sb[:], o_ps[:],
                                             mybir.ActivationFunctionType.Identity,
                                             bias=bias_eff[:, jd:jd + 1],
                                             scale=recip[:, jd:jd + 1])
                        nc.sync.dma_start(
                            out=out_ext[n, jd * 128:(jd + 1) * 128,
                                        ch * OC:(ch + 1) * OC],
                            in_=o_sb[:])

    nc.compile()
    return nc


_NC_CACHE = None


def kernel(**inputs: np.ndarray) -> np.ndarray:
    global _NC_CACHE
    from concourse.bass_utils import run_bass_kernel_spmd

    batch = np.asarray(inputs["batch_flat"], dtype=np.float32)
    Wq = np.asarray(inputs["Wq"], dtype=np.float32)
    bq = np.asarray(inputs["bq"], dtype=np.float32)
    Wk = np.asarray(inputs["Wk"], dtype=np.float32)
    bk = np.asarray(inputs["bk"], dtype=np.float32)
    Wv = np.asarray(inputs["Wv"], dtype=np.float32)
    bv = np.asarray(inputs["bv"], dtype=np.float32)

    if _NC_CACHE is None:
        _NC_CACHE = _build_nc()
    nc = _NC_CACHE

    x_r = _f32r_round(batch)
    wqT = _f32r_round(np.concatenate([Wq.T, bq[None, :]], axis=0))
    wkT = _f32r_round(np.concatenate([Wk.T, bk[None, :]], axis=0))
    wv = _f32r_round(Wv)
    bvb = np.ascontiguousarray(np.tile(_f32r_round(bv)[None, :], (128, 1)))
    ident = np.eye(128, dtype=np.float32)

    in_maps = []
    for c in range(NCORES):
        in_maps.append({
            "x": np.ascontiguousarray(x_r[c * NPC:(c + 1) * NPC]),
            "wqT": wqT, "wkT": wkT, "wv": wv, "bvb": bvb, "ident": ident,
            "bkb": np.ascontiguousarray(np.tile(wkT[C:C + 1, :], (128, 1))),
        })
    r = run_bass_kernel_spmd(nc, in_maps, core_ids=list(range(NCORES)))
    out = np.concatenate([r.results[c]["out"] for c in range(NCORES)], axis=0)
    return out.astype(np.float32)
